# revision 22
# baseline (speedup 1.0000x reference)
import dataclasses
import os

import numpy as np
import ml_dtypes

from concourse import bass, bass_utils, mybir

bf16 = ml_dtypes.bfloat16
f8 = ml_dtypes.float8_e4m3

# Problem constants (hardcoded: kernel.py must be self-contained)
D = 64
K = D * (D - 1) // 2     # 2016 triu vec length
S = 2048                 # slot-packed length: 32 slots x 64
M = D * D                # 4096 flat matrix
B = 8192
NCORES = 8
RB = B // NCORES         # 1024 rows per core
P = 128                  # partitions
NT = RB // P             # 8 row-tiles per core
G = 2                    # tiles per compute group
NG = NT // G             # 4 groups
ETA = 0.05
RADIUS = 0.693
SA = 64.0                # fp8 pre-scale for A_old
SD = 64.0                # fp8 pre-scale for dA
SCALE_OUT = 0.5 * ETA / (SA * SD)

_IU = np.triu_indices(D, 1)

LAST_EXEC_NS = None
_NC_CACHE = {}

# ---------------------------------------------------------------------------
# Packing tables. Slot s (s=0..31, width 64) holds strip s (row s, cols
# s+1..63: 63-s values) followed by strip 62-s (s+1 values); slot 31 is
# strip 31 + 32 pad. Total 2048 (vs 2016 vec) but every slot is fixed-width,
# which makes both unvec directions two rectangular strided copies.
# ---------------------------------------------------------------------------
_off = np.zeros(D, np.int64)
for _i in range(1, D):
    _off[_i] = _off[_i - 1] + (D - _i)

IDX_PACK = np.zeros(S, np.int64)
_valid = np.zeros(S, bool)
for _s in range(32):
    _L1 = 63 - _s
    IDX_PACK[_s * 64:_s * 64 + _L1] = _off[_s] + np.arange(_L1)
    _valid[_s * 64:_s * 64 + _L1] = True
    if _s < 31:
        _t = 62 - _s
        IDX_PACK[_s * 64 + _L1:_s * 64 + 64] = _off[_t] + np.arange(_s + 1)
        _valid[_s * 64 + _L1:_s * 64 + 64] = True

INV = np.zeros(K, np.int64)
INV[IDX_PACK[_valid]] = np.nonzero(_valid)[0]

PAD_COLS = np.nonzero(~_valid)[0]
# flat [64,64] indices of upper/lower mirror positions, in slot-packed order
IU_UP_PACK = (_IU[0] * D + _IU[1])[IDX_PACK]
IU_LO_PACK = (_IU[1] * D + _IU[0])[IDX_PACK]

# fp8 conversion LUTs (single-CPU host: every memory pass counts).
# encode: f32 -> f16 -> LUT16_F8 byte; decode: byte -> f32 (pre-scaled).
_all16 = np.arange(65536, dtype=np.uint16).view(np.float16)
with np.errstate(invalid="ignore", over="ignore"):
    LUT16_F8 = _all16.astype(np.float32).astype(f8).view(np.uint8)
LUT8_SCALED = (np.arange(256, dtype=np.uint8).view(f8).astype(np.float32)
               * SCALE_OUT)

_POOL = None


def _get_pool():
    global _POOL
    if _POOL is None:
        from concurrent.futures import ThreadPoolExecutor
        _POOL = ThreadPoolExecutor(max_workers=4)
    return _POOL


_WARM_FUT = None


def _warmup():
    """Build the jitted runner and push one dummy execution through the 8
    cores so NEFF load / executable load / allocations are all primed
    before the first real call."""
    import jax
    fn, mup_d, msel_d, wz_d = _get_runner()
    xz = np.zeros((NCORES * 2 * RB, S), f8)
    xd = jax.device_put(xz, mup_d.sharding)
    np.asarray(fn(xd, mup_d, msel_d, wz_d))
    return True


def _start_warmup():
    global _WARM_FUT
    if _WARM_FUT is None and os.environ.get("KERNEL_NO_WARMUP", "0") != "1":
        try:
            _WARM_FUT = _get_pool().submit(_warmup)
        except Exception:
            _WARM_FUT = None

# upper-triangular mask over flat [64,64], replicated across 128 partitions
_mup = np.zeros(M, np.float32)
_mup[_IU[0] * D + _IU[1]] = 1.0
MASK_UP = np.ascontiguousarray(np.broadcast_to(np.tile(_mup, G), (P, G * M))).astype(bf16)

# select mask: position (s,e) valid for part1 iff e < 63-s
_msel = np.zeros(S, np.float32)
for _s in range(32):
    _msel[_s * 64:_s * 64 + (63 - _s)] = 1.0
MASK_SEL = np.ascontiguousarray(np.broadcast_to(np.tile(_msel, G), (P, G * S))).astype(np.uint8)


def _ap(base, ap_dims, offset):
    """Custom strided AP over an SBUF tensor's flat [P, n] view."""
    return dataclasses.replace(base, ap=[base.ap[0]] + ap_dims, offset=offset)


def _build_nc():
    nc = bass.Bass()
    x = nc.dram_tensor("x", [2 * RB, S], mybir.dt.float8e4, kind="ExternalInput")
    mup = nc.dram_tensor("mup", [P, G * M], mybir.dt.bfloat16, kind="ExternalInput")
    msel = nc.dram_tensor("msel", [P, G * S], mybir.dt.uint8, kind="ExternalInput")
    wp = nc.dram_tensor("wp", [RB, S], mybir.dt.float8e4, kind="ExternalOutput")

    dt = mybir.dt.bfloat16
    mult = mybir.AluOpType.mult
    add = mybir.AluOpType.add
    sub = mybir.AluOpType.subtract

    with (
        nc.sbuf_tensor([P, G * M], dt) as sMu,
        nc.sbuf_tensor([P, G * S], mybir.dt.uint8) as sMs,
        nc.sbuf_tensor([P, G * S], mybir.dt.float8e4) as Vb,
        nc.sbuf_tensor([P, G * S], mybir.dt.float8e4) as Db,
        nc.sbuf_tensor([P, G * S], mybir.dt.float8e4) as Wv,
        nc.sbuf_tensor([P, G * M], dt) as UA,   # Up_A, later MAC tmp
        nc.sbuf_tensor([P, G * M], dt) as UD,   # Up_D, later P accumulator
        nc.sbuf_tensor([P, G * M], dt) as TA,   # A, later W
        nc.sbuf_tensor([P, G * M], dt) as TD,   # D
        nc.semaphore() as s_in,
        nc.semaphore() as s_mask,
        nc.semaphore() as s_v,
        nc.semaphore() as s_out,
        nc.Block() as block,
    ):
        # flat + structured views
        def tview(t, inner):  # [P, G*inner] -> [P, G, inner]
            return t[:, :].rearrange("p (g e) -> p g e", e=inner)

        Vb4 = Vb[:, :].rearrange("p (g s e) -> p g s e", s=32, e=64)
        Db4 = Db[:, :].rearrange("p (g s e) -> p g s e", s=32, e=64)
        Wv4 = Wv[:, :].rearrange("p (g s e) -> p g s e", s=32, e=64)

        A4 = TA[:, :].rearrange("p (g i j) -> p g i j", i=D, j=D)
        D4 = TD[:, :].rearrange("p (g i j) -> p g i j", i=D, j=D)
        P4 = UD[:, :].rearrange("p (g i j) -> p g i j", i=D, j=D)
        T4 = UA[:, :].rearrange("p (g i j) -> p g i j", i=D, j=D)

        # unvec target views on a flat base
        UAf = UA[:, :]
        UDf = UD[:, :]
        TAf = TA[:, :]

        @block.sync
        def _(sync):
            sync.dma_start(out=sMu[:, :], in_=mup[:, :]).then_inc(s_mask, 16)
            sync.dma_start(out=sMs[:, :], in_=msel[:, :]).then_inc(s_mask, 16)
            for g in range(NG):
                if g > 0:
                    sync.wait_ge(s_v, g)  # vector done with Vb/Db of group g-1
                rows = slice(g * G * P, (g + 1) * G * P)
                rows_d = slice(RB + g * G * P, RB + (g + 1) * G * P)
                sync.dma_start(
                    out=tview(Vb, S),
                    in_=x[rows, :].rearrange("(g p) e -> p g e", p=P),
                ).then_inc(s_in, 16)
                sync.dma_start(
                    out=tview(Db, S),
                    in_=x[rows_d, :].rearrange("(g p) e -> p g e", p=P),
                ).then_inc(s_in, 16)

        @block.vector
        def _(vector):
            def dr():
                vector.drain()

            def unvec(upf, up_struct, src4):
                # memset; part2 (rows 31..62 full-width; strip-s dup into
                # lower, masked later); then part1 (rows 0..31 upper; also
                # fixes row 31 upper over part2's slot-31 pad garbage).
                vector.memset(upf, 0.0)
                dr()
                out_p2 = up_struct[:, :, 31:63, :]          # [p g 32 64] rows 31..62
                in_p2 = src4[:, :, 31::-1, :]               # slots 31..0
                vector.tensor_copy(out_p2, in_p2)
                dr()
                out_p1 = _ap(upf, [[M, G], [65, 32], [1, 64]], 1)
                in_p1 = src4
                vector.tensor_copy(out_p1, in_p1)
                dr()
                # mask to strictly-upper
                vector.tensor_tensor(upf, upf, sMu[:, :], mult)
                dr()

            UPA = UA[:, :].rearrange("p (g r e) -> p g r e", r=D, e=D)
            UPD = UD[:, :].rearrange("p (g r e) -> p g r e", r=D, e=D)

            vector.wait_ge(s_mask, 32)
            for g in range(NG):
                vector.wait_ge(s_in, 32 * (g + 1))
                if g > 0:
                    vector.wait_ge(s_out, 16 * g)  # prior store drained

                unvec(UA[:, :], UPA, Vb4)
                # A = Up - Up^T
                ua = UA[:, :].rearrange("p (g i j) -> p g i j", i=D, j=D)
                vector.tensor_tensor(A4, ua, ua.transpose([0, 1, 3, 2]), sub)
                dr()
                unvec(UD[:, :], UPD, Db4)
                ud = UD[:, :].rearrange("p (g i j) -> p g i j", i=D, j=D)
                vector.tensor_tensor(D4, ud, ud.transpose([0, 1, 3, 2]), sub)
                dr()

                # MAC: P = A @ D, accumulated over k. UD is dead -> P, UA -> tmp.
                a0 = A4[:, :, :, 0].unsqueeze(3).broadcast_to([P, G, D, D])
                d0 = D4[:, :, 0, :].unsqueeze(2).broadcast_to([P, G, D, D])
                vector.tensor_tensor(P4, a0, d0, mult)
                dr()
                with vector.Fori(1, D) as k:
                    ak = A4[:, :, :, k].unsqueeze(3).broadcast_to([P, G, D, D])
                    dk = D4[:, :, k, :].unsqueeze(2).broadcast_to([P, G, D, D])
                    vector.tensor_tensor(T4, ak, dk, mult)
                    dr()
                    vector.tensor_tensor(P4, P4, T4, add)
                    dr()

                # W = P - P^T -> TA (A dead). Host applies 0.5*eta and
                # the fp8 input scales; values here are ~4096x the true
                # bracket, comfortably inside fp8-e4m3 range.
                vector.tensor_tensor(A4, P4, P4.transpose([0, 1, 3, 2]), sub)
                dr()
                # extract to slots: t2 -> Wv (fp8), t1 -> Vb (dead, fp8),
                # predicated merge
                in_t2 = _ap(TAf, [[M, G], [-64, 32], [1, 64]], 3968)
                vector.tensor_copy(Wv4, in_t2)
                in_t1 = _ap(TAf, [[M, G], [65, 32], [1, 64]], 1)
                vector.tensor_copy(Vb4, in_t1)
                dr()
                vector.copy_predicated(
                    Wv[:, :], sMs[:, :], Vb[:, :]
                ).then_inc(s_v, 1)

        @block.scalar
        def _(scalar):
            for g in range(NG):
                scalar.wait_ge(s_v, g + 1)
                rows = slice(g * G * P, (g + 1) * G * P)
                scalar.dma_start(
                    out=wp[rows, :].rearrange("(g p) e -> p g e", p=P),
                    in_=tview(Wv, S),
                ).then_inc(s_out, 16)

    return nc


def _get_runner():
    """Build (once) a cached jitted SPMD executor with device-resident masks
    and output-zero buffer. Re-jitting per call (as run_bass_kernel_spmd
    does) costs seconds of XLA compile + buffer churn per invocation."""
    if "fn" in _NC_CACHE:
        return _NC_CACHE["fn"]
    import jax
    import jax.numpy as jnp
    from jax.sharding import Mesh, NamedSharding, PartitionSpec
    from jax.experimental.shard_map import shard_map
    from concourse import bass2jax

    try:
        jax.config.update("jax_compilation_cache_dir", "/root/.jax_cache")
        jax.config.update("jax_persistent_cache_min_entry_size_bytes", -1)
        jax.config.update("jax_persistent_cache_min_compile_time_secs", 0.0)
    except Exception:
        pass

    bass2jax.install_neuronx_cc_hook()
    if "nc" not in _NC_CACHE:
        _NC_CACHE["nc"] = _build_nc()
    nc = _NC_CACHE["nc"]

    out_avals = (jax.core.ShapedArray((RB, S), jnp.float8_e4m3),)
    in_names = ("x", "mup", "msel", "wp", "partition_id")

    def _body(x_, mu_, ms_, wz_):
        outs = bass2jax._bass_exec_p.bind(
            x_, mu_, ms_, wz_, bass2jax.partition_id_tensor(),
            out_avals=out_avals,
            in_names=in_names,
            out_names=("wp",),
            lowering_input_output_aliases=(),
            sim_require_finite=True,
            sim_require_nnan=True,
            nc=nc,
        )
        return outs[0]

    devices = jax.devices()[:NCORES]
    _NC_CACHE["devices"] = devices
    mesh = Mesh(np.asarray(devices), ("core",))
    spec = PartitionSpec("core")
    sh = NamedSharding(mesh, spec)
    sm = shard_map(
        _body, mesh=mesh, in_specs=(spec,) * 4, out_specs=spec, check_rep=False
    )
    fn = jax.jit(sm, in_shardings=(sh,) * 4, out_shardings=sh)
    mup_d = jax.device_put(np.tile(MASK_UP, (NCORES, 1)), sh)
    msel_d = jax.device_put(np.tile(MASK_SEL, (NCORES, 1)), sh)
    wz_d = jax.device_put(np.zeros((NCORES * RB, S), f8), sh)
    _NC_CACHE["fn"] = (fn, mup_d, msel_d, wz_d)
    return _NC_CACHE["fn"]


def _exact_w_rows(vrows, drows):
    """Exact reference math (float64) for rows the cheap bounds can't settle.
    vrows: [n, K] f32 triu-vec of A_old; drows: [n, K] f32 triu-vec of dA.
    Returns w rows [n, K] f32 with w = vec(A_new) - vrows."""
    n = vrows.shape[0]
    A = np.zeros((n, D, D), np.float64)
    A[:, _IU[0], _IU[1]] = vrows.astype(np.float64)
    A -= A.transpose(0, 2, 1)
    dA = np.zeros((n, D, D), np.float64)
    dA[:, _IU[0], _IU[1]] = drows.astype(np.float64)
    dA -= dA.transpose(0, 2, 1)
    s_old = np.linalg.svd(A, compute_uv=False)[:, 0:1, None]
    s_del = np.linalg.svd(ETA * dA, compute_uv=False)[:, 0:1, None]
    avail = np.clip(RADIUS - s_old, 1e-8, None)
    scale = np.minimum(avail / (s_del + 1e-8), 1.0)
    dAs = dA * scale
    A_new = A + ETA * dAs + 0.5 * ETA * (np.matmul(A, dAs) - np.matmul(dAs, A))
    A_new = 0.5 * (A_new - A_new.transpose(0, 2, 1))
    s_new = np.linalg.svd(A_new, compute_uv=False)[:, 0:1, None]
    A_new = A_new * np.minimum(RADIUS / (s_new + 1e-8), 1.0)
    return (A_new[:, _IU[0], _IU[1]] - vrows.astype(np.float64)).astype(np.float32)


def kernel(**inputs):
    global LAST_EXEC_NS
    import time as _time
    _tm = os.environ.get("KERNEL_TIMING", "0") == "1"
    _ts = _time.time()

    def _lap(tag):
        nonlocal _ts
        if _tm:
            now = _time.time()
            print(f"  [kt] {tag}: {now - _ts:.3f}s", flush=True)
            _ts = now

    fib = np.ascontiguousarray(inputs["fiber_vectors"], dtype=np.float32)
    uid = np.asarray(inputs["user_ids"], dtype=np.int64)
    delta = np.ascontiguousarray(inputs["delta_A"], dtype=np.float32)

    trace = os.environ.get("KERNEL_TRACE", "0") == "1"
    pool = _get_pool()
    if not trace:
        import jax
        global _WARM_FUT
        if _WARM_FUT is not None:
            try:
                _WARM_FUT.result()
            except Exception:
                pass
            _WARM_FUT = None
        fn, mup_d, msel_d, wz_d = _get_runner()
        sh = mup_d.sharding
        devices = _NC_CACHE["devices"]
    _lap("setup")

    # serial per-core pack (1 host CPU); each shard's H2D streams in the
    # background while the next core is packed
    vpk = np.empty((B, S), np.float32)      # packed A_old rows (f32)
    dvec_pk = np.empty((B, S), np.float32)  # packed dA rows (f32)
    xg = np.empty((NCORES, 2, RB, S), f8)
    xg_u8 = xg.view(np.uint8)
    delta_flat = delta.reshape(B, M)
    col_pack = IDX_PACK[None, :]
    shards = [None] * NCORES

    for c in range(NCORES):
        lo, hi = c * RB, (c + 1) * RB
        vp_c = fib[uid[lo:hi, None], col_pack]
        vp_c[:, PAD_COLS] = 0.0
        vpk[lo:hi] = vp_c
        xg_u8[c, 0] = LUT16_F8[
            (vp_c * SA).astype(np.float16).view(np.uint16)]
        dfc = delta_flat[lo:hi]
        dpk_c = np.take(dfc, IU_UP_PACK, axis=1)
        dpk_c -= np.take(dfc, IU_LO_PACK, axis=1)
        dpk_c *= 0.5
        dpk_c[:, PAD_COLS] = 0.0
        dvec_pk[lo:hi] = dpk_c
        xg_u8[c, 1] = LUT16_F8[
            (dpk_c * SD).astype(np.float16).view(np.uint16)]
        if not trace:
            try:
                shards[c] = jax.device_put(xg[c].reshape(2 * RB, S), devices[c])
            except Exception:
                shards[c] = None
    _lap("pack+stream-put")

    fut_wpk = None
    if not trace:
        try:
            assert all(s is not None for s in shards)
            xd = jax.make_array_from_single_device_arrays(
                (NCORES * 2 * RB, S), sh, shards
            )
            outg = fn(xd, mup_d, msel_d, wz_d)   # async device pipeline
            fut_wpk = pool.submit(np.asarray, outg)  # network wait, no CPU
        except Exception:
            fut_wpk = None
        _lap("fn launch")

    # ---- overlap window: host work while device + D2H run ----
    # Frobenius sufficient conditions; rows they cannot settle go through
    # the exact host path (norms taken before the in-place base add below)
    fro_A = np.sqrt(2.0) * np.linalg.norm(vpk, axis=1)
    fro_dAe = ETA * np.sqrt(2.0) * np.linalg.norm(dvec_pk, axis=1)
    hard = ((RADIUS - fro_A) < fro_dAe + 1e-6) | (
        fro_A + fro_dAe + fro_A * fro_dAe > RADIUS - 1e-6
    )
    if hard.any():
        V_h = fib[uid[np.nonzero(hard)[0]]]
        hard_w = _exact_w_rows(V_h, dvec_pk[hard][:, INV])
    else:
        V_h = hard_w = None

    # base rows in packed space: vpk += eta * dvec_pk (in place)
    dvec_pk *= ETA
    vpk += dvec_pk

    out = fib.copy()
    _lap("overlap work")

    if trace:
        if "nc" not in _NC_CACHE:
            _NC_CACHE["nc"] = _build_nc()
        in_maps = []
        xgc = xg.reshape(NCORES, 2 * RB, S)
        for c in range(NCORES):
            in_maps.append({
                "x": xgc[c], "mup": MASK_UP, "msel": MASK_SEL,
            })
        res = bass_utils.run_bass_kernel_spmd(
            _NC_CACHE["nc"], in_maps, core_ids=list(range(NCORES)), trace=True,
        )
        LAST_EXEC_NS = res.exec_time_ns
        wpk = np.concatenate(
            [np.asarray(res.results[c]["wp"]) for c in range(NCORES)], axis=0
        )
    else:
        wpk = None
        if fut_wpk is not None:
            try:
                wpk = fut_wpk.result()
            except Exception:
                wpk = None
        if wpk is None:
            # device hiccup (e.g. exec-unit recovery): retry synchronously
            last_err = None
            for attempt in range(3):
                try:
                    _time.sleep(2.0 * attempt)
                    sh2 = [
                        jax.device_put(xg[c].reshape(2 * RB, S), devices[c])
                        for c in range(NCORES)
                    ]
                    xd2 = jax.make_array_from_single_device_arrays(
                        (NCORES * 2 * RB, S), sh, sh2
                    )
                    wpk = np.asarray(fn(xd2, mup_d, msel_d, wz_d))
                    break
                except Exception as e:
                    last_err = e
                    wpk = None
            if wpk is None:
                raise last_err
        LAST_EXEC_NS = None
        _lap("wait+D2H")

    # rows_new(packed) = base + scaled bracket; unpack via INV and scatter
    vpk += LUT8_SCALED[wpk.view(np.uint8)]
    out[uid] = vpk[:, INV]
    _lap("post")

    if hard_w is not None:
        out[uid[hard]] = V_h + hard_w
    return out


_start_warmup()


# revision 24
# speedup vs baseline: 2.2870x; 2.2870x over previous
import dataclasses
import os

import numpy as np
import ml_dtypes

from concourse import bass, bass_utils, mybir

bf16 = ml_dtypes.bfloat16
f8 = ml_dtypes.float8_e4m3

# Problem constants (hardcoded: kernel.py must be self-contained)
D = 64
K = D * (D - 1) // 2     # 2016 triu vec length
S = 2048                 # slot-packed length: 32 slots x 64
M = D * D                # 4096 flat matrix
B = 8192
NCORES = 8
RB = B // NCORES         # 1024 rows per core
P = 128                  # partitions
NT = RB // P             # 8 row-tiles per core
G = 2                    # tiles per compute group
NG = NT // G             # 4 groups
ETA = 0.05
RADIUS = 0.693
SA = 64.0                # fp8 pre-scale for A_old
SD = 64.0                # fp8 pre-scale for dA
SCALE_OUT = 0.5 * ETA / (SA * SD)

_IU = np.triu_indices(D, 1)

LAST_EXEC_NS = None
_NC_CACHE = {}

# ---------------------------------------------------------------------------
# Packing tables. Slot s (s=0..31, width 64) holds strip s (row s, cols
# s+1..63: 63-s values) followed by strip 62-s (s+1 values); slot 31 is
# strip 31 + 32 pad. Total 2048 (vs 2016 vec) but every slot is fixed-width,
# which makes both unvec directions two rectangular strided copies.
# ---------------------------------------------------------------------------
_off = np.zeros(D, np.int64)
for _i in range(1, D):
    _off[_i] = _off[_i - 1] + (D - _i)

IDX_PACK = np.zeros(S, np.int64)
_valid = np.zeros(S, bool)
for _s in range(32):
    _L1 = 63 - _s
    IDX_PACK[_s * 64:_s * 64 + _L1] = _off[_s] + np.arange(_L1)
    _valid[_s * 64:_s * 64 + _L1] = True
    if _s < 31:
        _t = 62 - _s
        IDX_PACK[_s * 64 + _L1:_s * 64 + 64] = _off[_t] + np.arange(_s + 1)
        _valid[_s * 64 + _L1:_s * 64 + 64] = True

INV = np.zeros(K, np.int64)
INV[IDX_PACK[_valid]] = np.nonzero(_valid)[0]

PAD_COLS = np.nonzero(~_valid)[0]
# flat [64,64] indices of upper/lower mirror positions, in slot-packed order
IU_UP_PACK = (_IU[0] * D + _IU[1])[IDX_PACK]
IU_LO_PACK = (_IU[1] * D + _IU[0])[IDX_PACK]

# fp8 conversion LUTs (single-CPU host: every memory pass counts).
# encode: f32 -> f16 -> LUT16_F8 byte; decode: byte -> f32 (pre-scaled).
_all16 = np.arange(65536, dtype=np.uint16).view(np.float16)
with np.errstate(invalid="ignore", over="ignore"):
    LUT16_F8 = _all16.astype(np.float32).astype(f8).view(np.uint8)
LUT8_SCALED = (np.arange(256, dtype=np.uint8).view(f8).astype(np.float32)
               * SCALE_OUT)

_POOL = None


def _get_pool():
    global _POOL
    if _POOL is None:
        from concurrent.futures import ThreadPoolExecutor
        _POOL = ThreadPoolExecutor(max_workers=4)
    return _POOL


_WARM_FUT = None


def _scratch():
    sc = _NC_CACHE.get("scratch")
    if sc is None:
        sc = {
            "vpk": np.empty((B, S), np.float32),
            "dvec_pk": np.empty((B, S), np.float32),
            "xg": np.empty((NCORES, 2, RB, S), f8),
            "rows": np.empty((B, K), np.float32),
            "brk": np.empty((B, S), np.float32),
        }
        _NC_CACHE["scratch"] = sc
    return sc


def _warmup():
    """Build the jitted runner and push one dummy execution through the 8
    cores so NEFF load / executable load / allocations are all primed
    before the first real call; pre-fault the scratch buffers."""
    import jax
    fn, mup_d, msel_d, wz_d = _get_runner()
    for a in _scratch().values():
        a.fill(0)
    xz = np.zeros((NCORES * 2 * RB, S), f8)
    xd = jax.device_put(xz, mup_d.sharding)
    np.asarray(fn(xd, mup_d, msel_d, wz_d))
    return True


def _start_warmup():
    global _WARM_FUT
    if _WARM_FUT is None and os.environ.get("KERNEL_NO_WARMUP", "0") != "1":
        try:
            _WARM_FUT = _get_pool().submit(_warmup)
        except Exception:
            _WARM_FUT = None

# upper-triangular mask over flat [64,64], replicated across 128 partitions
_mup = np.zeros(M, np.float32)
_mup[_IU[0] * D + _IU[1]] = 1.0
MASK_UP = np.ascontiguousarray(np.broadcast_to(np.tile(_mup, G), (P, G * M))).astype(bf16)

# select mask: position (s,e) valid for part1 iff e < 63-s
_msel = np.zeros(S, np.float32)
for _s in range(32):
    _msel[_s * 64:_s * 64 + (63 - _s)] = 1.0
MASK_SEL = np.ascontiguousarray(np.broadcast_to(np.tile(_msel, G), (P, G * S))).astype(np.uint8)


def _ap(base, ap_dims, offset):
    """Custom strided AP over an SBUF tensor's flat [P, n] view."""
    return dataclasses.replace(base, ap=[base.ap[0]] + ap_dims, offset=offset)


def _build_nc():
    nc = bass.Bass()
    x = nc.dram_tensor("x", [2 * RB, S], mybir.dt.float8e4, kind="ExternalInput")
    mup = nc.dram_tensor("mup", [P, G * M], mybir.dt.bfloat16, kind="ExternalInput")
    msel = nc.dram_tensor("msel", [P, G * S], mybir.dt.uint8, kind="ExternalInput")
    wp = nc.dram_tensor("wp", [RB, S], mybir.dt.float8e4, kind="ExternalOutput")

    dt = mybir.dt.bfloat16
    mult = mybir.AluOpType.mult
    add = mybir.AluOpType.add
    sub = mybir.AluOpType.subtract

    with (
        nc.sbuf_tensor([P, G * M], dt) as sMu,
        nc.sbuf_tensor([P, G * S], mybir.dt.uint8) as sMs,
        nc.sbuf_tensor([P, G * S], mybir.dt.float8e4) as Vb,
        nc.sbuf_tensor([P, G * S], mybir.dt.float8e4) as Db,
        nc.sbuf_tensor([P, G * S], mybir.dt.float8e4) as Wv,
        nc.sbuf_tensor([P, G * M], dt) as UA,   # Up_A, later MAC tmp
        nc.sbuf_tensor([P, G * M], dt) as UD,   # Up_D, later P accumulator
        nc.sbuf_tensor([P, G * M], dt) as TA,   # A, later W
        nc.sbuf_tensor([P, G * M], dt) as TD,   # D
        nc.semaphore() as s_in,
        nc.semaphore() as s_mask,
        nc.semaphore() as s_v,
        nc.semaphore() as s_out,
        nc.Block() as block,
    ):
        # flat + structured views
        def tview(t, inner):  # [P, G*inner] -> [P, G, inner]
            return t[:, :].rearrange("p (g e) -> p g e", e=inner)

        Vb4 = Vb[:, :].rearrange("p (g s e) -> p g s e", s=32, e=64)
        Db4 = Db[:, :].rearrange("p (g s e) -> p g s e", s=32, e=64)
        Wv4 = Wv[:, :].rearrange("p (g s e) -> p g s e", s=32, e=64)

        A4 = TA[:, :].rearrange("p (g i j) -> p g i j", i=D, j=D)
        D4 = TD[:, :].rearrange("p (g i j) -> p g i j", i=D, j=D)
        P4 = UD[:, :].rearrange("p (g i j) -> p g i j", i=D, j=D)
        T4 = UA[:, :].rearrange("p (g i j) -> p g i j", i=D, j=D)

        # unvec target views on a flat base
        UAf = UA[:, :]
        UDf = UD[:, :]
        TAf = TA[:, :]

        @block.sync
        def _(sync):
            sync.dma_start(out=sMu[:, :], in_=mup[:, :]).then_inc(s_mask, 16)
            sync.dma_start(out=sMs[:, :], in_=msel[:, :]).then_inc(s_mask, 16)
            for g in range(NG):
                if g > 0:
                    sync.wait_ge(s_v, g)  # vector done with Vb/Db of group g-1
                rows = slice(g * G * P, (g + 1) * G * P)
                rows_d = slice(RB + g * G * P, RB + (g + 1) * G * P)
                sync.dma_start(
                    out=tview(Vb, S),
                    in_=x[rows, :].rearrange("(g p) e -> p g e", p=P),
                ).then_inc(s_in, 16)
                sync.dma_start(
                    out=tview(Db, S),
                    in_=x[rows_d, :].rearrange("(g p) e -> p g e", p=P),
                ).then_inc(s_in, 16)

        @block.vector
        def _(vector):
            def dr():
                vector.drain()

            def unvec(upf, up_struct, src4):
                # memset; part2 (rows 31..62 full-width; strip-s dup into
                # lower, masked later); then part1 (rows 0..31 upper; also
                # fixes row 31 upper over part2's slot-31 pad garbage).
                vector.memset(upf, 0.0)
                dr()
                out_p2 = up_struct[:, :, 31:63, :]          # [p g 32 64] rows 31..62
                in_p2 = src4[:, :, 31::-1, :]               # slots 31..0
                vector.tensor_copy(out_p2, in_p2)
                dr()
                out_p1 = _ap(upf, [[M, G], [65, 32], [1, 64]], 1)
                in_p1 = src4
                vector.tensor_copy(out_p1, in_p1)
                dr()
                # mask to strictly-upper
                vector.tensor_tensor(upf, upf, sMu[:, :], mult)
                dr()

            UPA = UA[:, :].rearrange("p (g r e) -> p g r e", r=D, e=D)
            UPD = UD[:, :].rearrange("p (g r e) -> p g r e", r=D, e=D)

            vector.wait_ge(s_mask, 32)
            for g in range(NG):
                vector.wait_ge(s_in, 32 * (g + 1))
                if g > 0:
                    vector.wait_ge(s_out, 16 * g)  # prior store drained

                unvec(UA[:, :], UPA, Vb4)
                # A = Up - Up^T
                ua = UA[:, :].rearrange("p (g i j) -> p g i j", i=D, j=D)
                vector.tensor_tensor(A4, ua, ua.transpose([0, 1, 3, 2]), sub)
                dr()
                unvec(UD[:, :], UPD, Db4)
                ud = UD[:, :].rearrange("p (g i j) -> p g i j", i=D, j=D)
                vector.tensor_tensor(D4, ud, ud.transpose([0, 1, 3, 2]), sub)
                dr()

                # MAC: P = A @ D, accumulated over k. UD is dead -> P, UA -> tmp.
                a0 = A4[:, :, :, 0].unsqueeze(3).broadcast_to([P, G, D, D])
                d0 = D4[:, :, 0, :].unsqueeze(2).broadcast_to([P, G, D, D])
                vector.tensor_tensor(P4, a0, d0, mult)
                dr()
                with vector.Fori(1, D) as k:
                    ak = A4[:, :, :, k].unsqueeze(3).broadcast_to([P, G, D, D])
                    dk = D4[:, :, k, :].unsqueeze(2).broadcast_to([P, G, D, D])
                    vector.tensor_tensor(T4, ak, dk, mult)
                    dr()
                    vector.tensor_tensor(P4, P4, T4, add)
                    dr()

                # W = P - P^T -> TA (A dead). Host applies 0.5*eta and
                # the fp8 input scales; values here are ~4096x the true
                # bracket, comfortably inside fp8-e4m3 range.
                vector.tensor_tensor(A4, P4, P4.transpose([0, 1, 3, 2]), sub)
                dr()
                # extract to slots: t2 -> Wv (fp8), t1 -> Vb (dead, fp8),
                # predicated merge
                in_t2 = _ap(TAf, [[M, G], [-64, 32], [1, 64]], 3968)
                vector.tensor_copy(Wv4, in_t2)
                in_t1 = _ap(TAf, [[M, G], [65, 32], [1, 64]], 1)
                vector.tensor_copy(Vb4, in_t1)
                dr()
                vector.copy_predicated(
                    Wv[:, :], sMs[:, :], Vb[:, :]
                ).then_inc(s_v, 1)

        @block.scalar
        def _(scalar):
            for g in range(NG):
                scalar.wait_ge(s_v, g + 1)
                rows = slice(g * G * P, (g + 1) * G * P)
                scalar.dma_start(
                    out=wp[rows, :].rearrange("(g p) e -> p g e", p=P),
                    in_=tview(Wv, S),
                ).then_inc(s_out, 16)

    return nc


def _get_runner():
    """Build (once) a cached jitted SPMD executor with device-resident masks
    and output-zero buffer. Re-jitting per call (as run_bass_kernel_spmd
    does) costs seconds of XLA compile + buffer churn per invocation."""
    if "fn" in _NC_CACHE:
        return _NC_CACHE["fn"]
    import jax
    import jax.numpy as jnp
    from jax.sharding import Mesh, NamedSharding, PartitionSpec
    from jax.experimental.shard_map import shard_map
    from concourse import bass2jax

    try:
        jax.config.update("jax_compilation_cache_dir", "/root/.jax_cache")
        jax.config.update("jax_persistent_cache_min_entry_size_bytes", -1)
        jax.config.update("jax_persistent_cache_min_compile_time_secs", 0.0)
    except Exception:
        pass

    bass2jax.install_neuronx_cc_hook()
    if "nc" not in _NC_CACHE:
        _NC_CACHE["nc"] = _build_nc()
    nc = _NC_CACHE["nc"]

    out_avals = (jax.core.ShapedArray((RB, S), jnp.float8_e4m3),)
    in_names = ("x", "mup", "msel", "wp", "partition_id")

    def _body(x_, mu_, ms_, wz_):
        outs = bass2jax._bass_exec_p.bind(
            x_, mu_, ms_, wz_, bass2jax.partition_id_tensor(),
            out_avals=out_avals,
            in_names=in_names,
            out_names=("wp",),
            lowering_input_output_aliases=(),
            sim_require_finite=True,
            sim_require_nnan=True,
            nc=nc,
        )
        return outs[0]

    devices = jax.devices()[:NCORES]
    _NC_CACHE["devices"] = devices
    mesh = Mesh(np.asarray(devices), ("core",))
    spec = PartitionSpec("core")
    sh = NamedSharding(mesh, spec)
    sm = shard_map(
        _body, mesh=mesh, in_specs=(spec,) * 4, out_specs=spec, check_rep=False
    )
    fn = jax.jit(sm, in_shardings=(sh,) * 4, out_shardings=sh)
    mup_d = jax.device_put(np.tile(MASK_UP, (NCORES, 1)), sh)
    msel_d = jax.device_put(np.tile(MASK_SEL, (NCORES, 1)), sh)
    wz_d = jax.device_put(np.zeros((NCORES * RB, S), f8), sh)
    _NC_CACHE["fn"] = (fn, mup_d, msel_d, wz_d)
    return _NC_CACHE["fn"]


def _exact_w_rows(vrows, drows):
    """Exact reference math (float64) for rows the cheap bounds can't settle.
    vrows: [n, K] f32 triu-vec of A_old; drows: [n, K] f32 triu-vec of dA.
    Returns w rows [n, K] f32 with w = vec(A_new) - vrows."""
    n = vrows.shape[0]
    A = np.zeros((n, D, D), np.float64)
    A[:, _IU[0], _IU[1]] = vrows.astype(np.float64)
    A -= A.transpose(0, 2, 1)
    dA = np.zeros((n, D, D), np.float64)
    dA[:, _IU[0], _IU[1]] = drows.astype(np.float64)
    dA -= dA.transpose(0, 2, 1)
    s_old = np.linalg.svd(A, compute_uv=False)[:, 0:1, None]
    s_del = np.linalg.svd(ETA * dA, compute_uv=False)[:, 0:1, None]
    avail = np.clip(RADIUS - s_old, 1e-8, None)
    scale = np.minimum(avail / (s_del + 1e-8), 1.0)
    dAs = dA * scale
    A_new = A + ETA * dAs + 0.5 * ETA * (np.matmul(A, dAs) - np.matmul(dAs, A))
    A_new = 0.5 * (A_new - A_new.transpose(0, 2, 1))
    s_new = np.linalg.svd(A_new, compute_uv=False)[:, 0:1, None]
    A_new = A_new * np.minimum(RADIUS / (s_new + 1e-8), 1.0)
    return (A_new[:, _IU[0], _IU[1]] - vrows.astype(np.float64)).astype(np.float32)


def kernel(**inputs):
    global LAST_EXEC_NS
    import time as _time
    _tm = os.environ.get("KERNEL_TIMING", "0") == "1"
    _ts = _time.time()

    def _lap(tag):
        nonlocal _ts
        if _tm:
            now = _time.time()
            print(f"  [kt] {tag}: {now - _ts:.3f}s", flush=True)
            _ts = now

    fib = np.ascontiguousarray(inputs["fiber_vectors"], dtype=np.float32)
    uid = np.asarray(inputs["user_ids"], dtype=np.int64)
    delta = np.ascontiguousarray(inputs["delta_A"], dtype=np.float32)

    trace = os.environ.get("KERNEL_TRACE", "0") == "1"
    pool = _get_pool()
    if not trace:
        import jax
        global _WARM_FUT
        if _WARM_FUT is not None:
            try:
                _WARM_FUT.result()
            except Exception:
                pass
            _WARM_FUT = None
        fn, mup_d, msel_d, wz_d = _get_runner()
        sh = mup_d.sharding
        devices = _NC_CACHE["devices"]
    _lap("setup")

    # serial per-core pack (1 host CPU); each shard's H2D streams in the
    # background while the next core is packed
    sc = _scratch()
    vpk = sc["vpk"]          # packed A_old rows (f32)
    dvec_pk = sc["dvec_pk"]  # packed dA rows (f32)
    xg = sc["xg"]
    xg_u8 = xg.view(np.uint8)
    delta_flat = delta.reshape(B, M)
    col_pack = IDX_PACK[None, :]
    shards = [None] * NCORES

    for c in range(NCORES):
        lo, hi = c * RB, (c + 1) * RB
        vp_c = fib[uid[lo:hi, None], col_pack]
        vp_c[:, PAD_COLS] = 0.0
        vpk[lo:hi] = vp_c
        xg_u8[c, 0] = LUT16_F8[
            np.multiply(vp_c, SA, dtype=np.float16).view(np.uint16)]
        # dvec_pk holds (up - lo); the 0.5 skew factor is folded into the
        # downstream constants (encode scale, norms, base add, hard path)
        dfc = delta_flat[lo:hi]
        dsub = np.take(dfc, IU_UP_PACK, axis=1)
        dsub -= np.take(dfc, IU_LO_PACK, axis=1)
        dsub[:, PAD_COLS] = 0.0
        dvec_pk[lo:hi] = dsub
        xg_u8[c, 1] = LUT16_F8[
            np.multiply(dsub, 0.5 * SD, dtype=np.float16).view(np.uint16)]
        if not trace:
            try:
                shards[c] = jax.device_put(xg[c].reshape(2 * RB, S), devices[c])
            except Exception:
                shards[c] = None
    _lap("pack+stream-put")

    fut_wpk = None
    if not trace:
        try:
            assert all(s is not None for s in shards)
            xd = jax.make_array_from_single_device_arrays(
                (NCORES * 2 * RB, S), sh, shards
            )
            outg = fn(xd, mup_d, msel_d, wz_d)   # async device pipeline
            fut_wpk = pool.submit(np.asarray, outg)  # network wait, no CPU
        except Exception:
            fut_wpk = None
        _lap("fn launch")

    # ---- overlap window: host work while device + D2H run ----
    # Frobenius sufficient conditions; rows they cannot settle go through
    # the exact host path (norms taken before the in-place base add below)
    fro_A = np.sqrt(2.0) * np.linalg.norm(vpk, axis=1)
    fro_dAe = 0.5 * ETA * np.sqrt(2.0) * np.linalg.norm(dvec_pk, axis=1)
    hard = ((RADIUS - fro_A) < fro_dAe + 1e-6) | (
        fro_A + fro_dAe + fro_A * fro_dAe > RADIUS - 1e-6
    )
    if hard.any():
        V_h = fib[uid[np.nonzero(hard)[0]]]
        hard_w = _exact_w_rows(V_h, 0.5 * dvec_pk[hard][:, INV])
    else:
        V_h = hard_w = None

    # base rows in packed space: vpk += eta * 0.5 * (up - lo) (in place)
    dvec_pk *= 0.5 * ETA
    vpk += dvec_pk

    out = fib.copy()
    _lap("overlap work")

    if trace:
        if "nc" not in _NC_CACHE:
            _NC_CACHE["nc"] = _build_nc()
        in_maps = []
        xgc = xg.reshape(NCORES, 2 * RB, S)
        for c in range(NCORES):
            in_maps.append({
                "x": xgc[c], "mup": MASK_UP, "msel": MASK_SEL,
            })
        res = bass_utils.run_bass_kernel_spmd(
            _NC_CACHE["nc"], in_maps, core_ids=list(range(NCORES)), trace=True,
        )
        LAST_EXEC_NS = res.exec_time_ns
        wpk = np.concatenate(
            [np.asarray(res.results[c]["wp"]) for c in range(NCORES)], axis=0
        )
    else:
        wpk = None
        if fut_wpk is not None:
            try:
                wpk = fut_wpk.result()
            except Exception:
                wpk = None
        if wpk is None:
            # device hiccup (e.g. exec-unit recovery): retry synchronously
            last_err = None
            for attempt in range(3):
                try:
                    _time.sleep(2.0 * attempt)
                    sh2 = [
                        jax.device_put(xg[c].reshape(2 * RB, S), devices[c])
                        for c in range(NCORES)
                    ]
                    xd2 = jax.make_array_from_single_device_arrays(
                        (NCORES * 2 * RB, S), sh, sh2
                    )
                    wpk = np.asarray(fn(xd2, mup_d, msel_d, wz_d))
                    break
                except Exception as e:
                    last_err = e
                    wpk = None
            if wpk is None:
                raise last_err
        LAST_EXEC_NS = None
        _lap("wait+D2H")

    # rows_new(packed) = base + scaled bracket; unpack via INV and scatter
    np.take(LUT8_SCALED, wpk.view(np.uint8), out=sc["brk"])
    vpk += sc["brk"]
    np.take(vpk, INV, axis=1, out=sc["rows"])
    out[uid] = sc["rows"]
    _lap("post")

    if hard_w is not None:
        out[uid[hard]] = V_h + hard_w
    return out


_start_warmup()


# revision 25
# speedup vs baseline: 2.4597x; 1.0755x over previous
import dataclasses
import os

import numpy as np
import ml_dtypes

from concourse import bass, bass_utils, mybir

bf16 = ml_dtypes.bfloat16
f8 = ml_dtypes.float8_e4m3

# Problem constants (hardcoded: kernel.py must be self-contained)
D = 64
K = D * (D - 1) // 2     # 2016 triu vec length
S = 2048                 # slot-packed length: 32 slots x 64
M = D * D                # 4096 flat matrix
B = 8192
NCORES = 8
RB = B // NCORES         # 1024 rows per core
P = 128                  # partitions
NT = RB // P             # 8 row-tiles per core
G = 2                    # tiles per compute group
NG = NT // G             # 4 groups
ETA = 0.05
RADIUS = 0.693
SA = 64.0                # fp8 pre-scale for A_old
SD = 64.0                # fp8 pre-scale for dA
SCALE_OUT = 0.5 * ETA / (SA * SD)

_IU = np.triu_indices(D, 1)

LAST_EXEC_NS = None
_NC_CACHE = {}

# ---------------------------------------------------------------------------
# Packing tables. Slot s (s=0..31, width 64) holds strip s (row s, cols
# s+1..63: 63-s values) followed by strip 62-s (s+1 values); slot 31 is
# strip 31 + 32 pad. Total 2048 (vs 2016 vec) but every slot is fixed-width,
# which makes both unvec directions two rectangular strided copies.
# ---------------------------------------------------------------------------
_off = np.zeros(D, np.int64)
for _i in range(1, D):
    _off[_i] = _off[_i - 1] + (D - _i)

IDX_PACK = np.zeros(S, np.int64)
_valid = np.zeros(S, bool)
for _s in range(32):
    _L1 = 63 - _s
    IDX_PACK[_s * 64:_s * 64 + _L1] = _off[_s] + np.arange(_L1)
    _valid[_s * 64:_s * 64 + _L1] = True
    if _s < 31:
        _t = 62 - _s
        IDX_PACK[_s * 64 + _L1:_s * 64 + 64] = _off[_t] + np.arange(_s + 1)
        _valid[_s * 64 + _L1:_s * 64 + 64] = True

INV = np.zeros(K, np.int64)
INV[IDX_PACK[_valid]] = np.nonzero(_valid)[0]

PAD_COLS = np.nonzero(~_valid)[0]
# flat [64,64] indices of upper/lower mirror positions, in slot-packed order
IU_UP_PACK = (_IU[0] * D + _IU[1])[IDX_PACK]
IU_LO_PACK = (_IU[1] * D + _IU[0])[IDX_PACK]

# fp8 conversion LUTs (single-CPU host: every memory pass counts).
# encode: f32 -> f16 -> LUT16_F8 byte; decode: byte -> f32 (pre-scaled).
_all16 = np.arange(65536, dtype=np.uint16).view(np.float16)
with np.errstate(invalid="ignore", over="ignore"):
    LUT16_F8 = _all16.astype(np.float32).astype(f8).view(np.uint8)
LUT8_SCALED = (np.arange(256, dtype=np.uint8).view(f8).astype(np.float32)
               * SCALE_OUT)
# u16-pair decode LUT: one lookup yields two pre-scaled f32s (half the
# index traffic of the byte LUT)
_b16 = np.arange(65536, dtype=np.uint16)
LUT2_SCALED = np.ascontiguousarray(np.stack([
    (_b16 & 0xFF).astype(np.uint8).view(f8).astype(np.float32) * SCALE_OUT,
    (_b16 >> 8).astype(np.uint8).view(f8).astype(np.float32) * SCALE_OUT,
], axis=1))

_POOL = None


def _get_pool():
    global _POOL
    if _POOL is None:
        from concurrent.futures import ThreadPoolExecutor
        _POOL = ThreadPoolExecutor(max_workers=4)
    return _POOL


_WARM_FUT = None


def _scratch():
    sc = _NC_CACHE.get("scratch")
    if sc is None:
        sc = {
            "vpk": np.empty((B, S), np.float32),
            "dvec_pk": np.empty((B, S), np.float32),
            "xg": np.empty((NCORES, 2, RB, S), f8),
            "rows": np.empty((B, K), np.float32),
            "brk": np.empty((B, S), np.float32),
        }
        _NC_CACHE["scratch"] = sc
    return sc


def _warmup():
    """Build the jitted runner and push one dummy execution through the 8
    cores so NEFF load / executable load / allocations are all primed
    before the first real call; pre-fault the scratch buffers."""
    import jax
    fn, mup_d, msel_d, wz_d = _get_runner()
    for a in _scratch().values():
        a.fill(0)
    xz = np.zeros((NCORES * 2 * RB, S), f8)
    for attempt in range(3):
        try:
            xd = jax.device_put(xz, mup_d.sharding)
            np.asarray(fn(xd, mup_d, msel_d, wz_d))
            return True
        except Exception:
            import time as _t
            _t.sleep(2.0 * (attempt + 1))
    return False


def _start_warmup():
    global _WARM_FUT
    if _WARM_FUT is None and os.environ.get("KERNEL_NO_WARMUP", "0") != "1":
        try:
            _WARM_FUT = _get_pool().submit(_warmup)
        except Exception:
            _WARM_FUT = None

# upper-triangular mask over flat [64,64], replicated across 128 partitions
_mup = np.zeros(M, np.float32)
_mup[_IU[0] * D + _IU[1]] = 1.0
MASK_UP = np.ascontiguousarray(np.broadcast_to(np.tile(_mup, G), (P, G * M))).astype(bf16)

# select mask: position (s,e) valid for part1 iff e < 63-s
_msel = np.zeros(S, np.float32)
for _s in range(32):
    _msel[_s * 64:_s * 64 + (63 - _s)] = 1.0
MASK_SEL = np.ascontiguousarray(np.broadcast_to(np.tile(_msel, G), (P, G * S))).astype(np.uint8)


def _ap(base, ap_dims, offset):
    """Custom strided AP over an SBUF tensor's flat [P, n] view."""
    return dataclasses.replace(base, ap=[base.ap[0]] + ap_dims, offset=offset)


def _build_nc():
    nc = bass.Bass()
    x = nc.dram_tensor("x", [2 * RB, S], mybir.dt.float8e4, kind="ExternalInput")
    mup = nc.dram_tensor("mup", [P, G * M], mybir.dt.bfloat16, kind="ExternalInput")
    msel = nc.dram_tensor("msel", [P, G * S], mybir.dt.uint8, kind="ExternalInput")
    wp = nc.dram_tensor("wp", [RB, S], mybir.dt.float8e4, kind="ExternalOutput")

    dt = mybir.dt.bfloat16
    mult = mybir.AluOpType.mult
    add = mybir.AluOpType.add
    sub = mybir.AluOpType.subtract

    with (
        nc.sbuf_tensor([P, G * M], dt) as sMu,
        nc.sbuf_tensor([P, G * S], mybir.dt.uint8) as sMs,
        nc.sbuf_tensor([P, G * S], mybir.dt.float8e4) as Vb,
        nc.sbuf_tensor([P, G * S], mybir.dt.float8e4) as Db,
        nc.sbuf_tensor([P, G * S], mybir.dt.float8e4) as Wv,
        nc.sbuf_tensor([P, G * M], dt) as UA,   # Up_A, later MAC tmp
        nc.sbuf_tensor([P, G * M], dt) as UD,   # Up_D, later P accumulator
        nc.sbuf_tensor([P, G * M], dt) as TA,   # A, later W
        nc.sbuf_tensor([P, G * M], dt) as TD,   # D
        nc.semaphore() as s_in,
        nc.semaphore() as s_mask,
        nc.semaphore() as s_v,
        nc.semaphore() as s_out,
        nc.Block() as block,
    ):
        # flat + structured views
        def tview(t, inner):  # [P, G*inner] -> [P, G, inner]
            return t[:, :].rearrange("p (g e) -> p g e", e=inner)

        Vb4 = Vb[:, :].rearrange("p (g s e) -> p g s e", s=32, e=64)
        Db4 = Db[:, :].rearrange("p (g s e) -> p g s e", s=32, e=64)
        Wv4 = Wv[:, :].rearrange("p (g s e) -> p g s e", s=32, e=64)

        A4 = TA[:, :].rearrange("p (g i j) -> p g i j", i=D, j=D)
        D4 = TD[:, :].rearrange("p (g i j) -> p g i j", i=D, j=D)
        P4 = UD[:, :].rearrange("p (g i j) -> p g i j", i=D, j=D)
        T4 = UA[:, :].rearrange("p (g i j) -> p g i j", i=D, j=D)

        # unvec target views on a flat base
        UAf = UA[:, :]
        UDf = UD[:, :]
        TAf = TA[:, :]

        @block.sync
        def _(sync):
            sync.dma_start(out=sMu[:, :], in_=mup[:, :]).then_inc(s_mask, 16)
            sync.dma_start(out=sMs[:, :], in_=msel[:, :]).then_inc(s_mask, 16)
            for g in range(NG):
                if g > 0:
                    sync.wait_ge(s_v, g)  # vector done with Vb/Db of group g-1
                rows = slice(g * G * P, (g + 1) * G * P)
                rows_d = slice(RB + g * G * P, RB + (g + 1) * G * P)
                sync.dma_start(
                    out=tview(Vb, S),
                    in_=x[rows, :].rearrange("(g p) e -> p g e", p=P),
                ).then_inc(s_in, 16)
                sync.dma_start(
                    out=tview(Db, S),
                    in_=x[rows_d, :].rearrange("(g p) e -> p g e", p=P),
                ).then_inc(s_in, 16)

        @block.vector
        def _(vector):
            def dr():
                vector.drain()

            def unvec(upf, up_struct, src4):
                # memset; part2 (rows 31..62 full-width; strip-s dup into
                # lower, masked later); then part1 (rows 0..31 upper; also
                # fixes row 31 upper over part2's slot-31 pad garbage).
                vector.memset(upf, 0.0)
                dr()
                out_p2 = up_struct[:, :, 31:63, :]          # [p g 32 64] rows 31..62
                in_p2 = src4[:, :, 31::-1, :]               # slots 31..0
                vector.tensor_copy(out_p2, in_p2)
                dr()
                out_p1 = _ap(upf, [[M, G], [65, 32], [1, 64]], 1)
                in_p1 = src4
                vector.tensor_copy(out_p1, in_p1)
                dr()
                # mask to strictly-upper
                vector.tensor_tensor(upf, upf, sMu[:, :], mult)
                dr()

            UPA = UA[:, :].rearrange("p (g r e) -> p g r e", r=D, e=D)
            UPD = UD[:, :].rearrange("p (g r e) -> p g r e", r=D, e=D)

            vector.wait_ge(s_mask, 32)
            for g in range(NG):
                vector.wait_ge(s_in, 32 * (g + 1))
                if g > 0:
                    vector.wait_ge(s_out, 16 * g)  # prior store drained

                unvec(UA[:, :], UPA, Vb4)
                # A = Up - Up^T
                ua = UA[:, :].rearrange("p (g i j) -> p g i j", i=D, j=D)
                vector.tensor_tensor(A4, ua, ua.transpose([0, 1, 3, 2]), sub)
                dr()
                unvec(UD[:, :], UPD, Db4)
                ud = UD[:, :].rearrange("p (g i j) -> p g i j", i=D, j=D)
                vector.tensor_tensor(D4, ud, ud.transpose([0, 1, 3, 2]), sub)
                dr()

                # MAC: P = A @ D, accumulated over k. UD is dead -> P, UA -> tmp.
                a0 = A4[:, :, :, 0].unsqueeze(3).broadcast_to([P, G, D, D])
                d0 = D4[:, :, 0, :].unsqueeze(2).broadcast_to([P, G, D, D])
                vector.tensor_tensor(P4, a0, d0, mult)
                dr()
                with vector.Fori(1, D) as k:
                    ak = A4[:, :, :, k].unsqueeze(3).broadcast_to([P, G, D, D])
                    dk = D4[:, :, k, :].unsqueeze(2).broadcast_to([P, G, D, D])
                    vector.tensor_tensor(T4, ak, dk, mult)
                    dr()
                    vector.tensor_tensor(P4, P4, T4, add)
                    dr()

                # W = P - P^T -> TA (A dead). Host applies 0.5*eta and
                # the fp8 input scales; values here are ~4096x the true
                # bracket, comfortably inside fp8-e4m3 range.
                vector.tensor_tensor(A4, P4, P4.transpose([0, 1, 3, 2]), sub)
                dr()
                # extract to slots: t2 -> Wv (fp8), t1 -> Vb (dead, fp8),
                # predicated merge
                in_t2 = _ap(TAf, [[M, G], [-64, 32], [1, 64]], 3968)
                vector.tensor_copy(Wv4, in_t2)
                in_t1 = _ap(TAf, [[M, G], [65, 32], [1, 64]], 1)
                vector.tensor_copy(Vb4, in_t1)
                dr()
                vector.copy_predicated(
                    Wv[:, :], sMs[:, :], Vb[:, :]
                ).then_inc(s_v, 1)

        @block.scalar
        def _(scalar):
            for g in range(NG):
                scalar.wait_ge(s_v, g + 1)
                rows = slice(g * G * P, (g + 1) * G * P)
                scalar.dma_start(
                    out=wp[rows, :].rearrange("(g p) e -> p g e", p=P),
                    in_=tview(Wv, S),
                ).then_inc(s_out, 16)

    return nc


def _get_runner():
    """Build (once) a cached jitted SPMD executor with device-resident masks
    and output-zero buffer. Re-jitting per call (as run_bass_kernel_spmd
    does) costs seconds of XLA compile + buffer churn per invocation."""
    if "fn" in _NC_CACHE:
        return _NC_CACHE["fn"]
    import jax
    import jax.numpy as jnp
    from jax.sharding import Mesh, NamedSharding, PartitionSpec
    from jax.experimental.shard_map import shard_map
    from concourse import bass2jax

    try:
        jax.config.update("jax_compilation_cache_dir", "/root/.jax_cache")
        jax.config.update("jax_persistent_cache_min_entry_size_bytes", -1)
        jax.config.update("jax_persistent_cache_min_compile_time_secs", 0.0)
    except Exception:
        pass

    bass2jax.install_neuronx_cc_hook()
    if "nc" not in _NC_CACHE:
        _NC_CACHE["nc"] = _build_nc()
    nc = _NC_CACHE["nc"]

    out_avals = (jax.core.ShapedArray((RB, S), jnp.float8_e4m3),)
    in_names = ("x", "mup", "msel", "wp", "partition_id")

    def _body(x_, mu_, ms_, wz_):
        outs = bass2jax._bass_exec_p.bind(
            x_, mu_, ms_, wz_, bass2jax.partition_id_tensor(),
            out_avals=out_avals,
            in_names=in_names,
            out_names=("wp",),
            lowering_input_output_aliases=(),
            sim_require_finite=True,
            sim_require_nnan=True,
            nc=nc,
        )
        return outs[0]

    devices = jax.devices()[:NCORES]
    _NC_CACHE["devices"] = devices
    mesh = Mesh(np.asarray(devices), ("core",))
    spec = PartitionSpec("core")
    sh = NamedSharding(mesh, spec)
    sm = shard_map(
        _body, mesh=mesh, in_specs=(spec,) * 4, out_specs=spec, check_rep=False
    )
    fn = jax.jit(sm, in_shardings=(sh,) * 4, out_shardings=sh)
    mup_d = jax.device_put(np.tile(MASK_UP, (NCORES, 1)), sh)
    msel_d = jax.device_put(np.tile(MASK_SEL, (NCORES, 1)), sh)
    wz_d = jax.device_put(np.zeros((NCORES * RB, S), f8), sh)
    _NC_CACHE["fn"] = (fn, mup_d, msel_d, wz_d)
    return _NC_CACHE["fn"]


def _exact_w_rows(vrows, drows):
    """Exact reference math (float64) for rows the cheap bounds can't settle.
    vrows: [n, K] f32 triu-vec of A_old; drows: [n, K] f32 triu-vec of dA.
    Returns w rows [n, K] f32 with w = vec(A_new) - vrows."""
    n = vrows.shape[0]
    A = np.zeros((n, D, D), np.float64)
    A[:, _IU[0], _IU[1]] = vrows.astype(np.float64)
    A -= A.transpose(0, 2, 1)
    dA = np.zeros((n, D, D), np.float64)
    dA[:, _IU[0], _IU[1]] = drows.astype(np.float64)
    dA -= dA.transpose(0, 2, 1)
    s_old = np.linalg.svd(A, compute_uv=False)[:, 0:1, None]
    s_del = np.linalg.svd(ETA * dA, compute_uv=False)[:, 0:1, None]
    avail = np.clip(RADIUS - s_old, 1e-8, None)
    scale = np.minimum(avail / (s_del + 1e-8), 1.0)
    dAs = dA * scale
    A_new = A + ETA * dAs + 0.5 * ETA * (np.matmul(A, dAs) - np.matmul(dAs, A))
    A_new = 0.5 * (A_new - A_new.transpose(0, 2, 1))
    s_new = np.linalg.svd(A_new, compute_uv=False)[:, 0:1, None]
    A_new = A_new * np.minimum(RADIUS / (s_new + 1e-8), 1.0)
    return (A_new[:, _IU[0], _IU[1]] - vrows.astype(np.float64)).astype(np.float32)


def kernel(**inputs):
    global LAST_EXEC_NS
    import time as _time
    _tm = os.environ.get("KERNEL_TIMING", "0") == "1"
    _ts = _time.time()

    def _lap(tag):
        nonlocal _ts
        if _tm:
            now = _time.time()
            print(f"  [kt] {tag}: {now - _ts:.3f}s", flush=True)
            _ts = now

    fib = np.ascontiguousarray(inputs["fiber_vectors"], dtype=np.float32)
    uid = np.asarray(inputs["user_ids"], dtype=np.int64)
    delta = np.ascontiguousarray(inputs["delta_A"], dtype=np.float32)

    trace = os.environ.get("KERNEL_TRACE", "0") == "1"
    pool = _get_pool()
    if not trace:
        import jax
        global _WARM_FUT
        if _WARM_FUT is not None:
            try:
                _WARM_FUT.result()
            except Exception:
                pass
            _WARM_FUT = None
        fn, mup_d, msel_d, wz_d = _get_runner()
        sh = mup_d.sharding
        devices = _NC_CACHE["devices"]
    _lap("setup")

    # serial per-core pack (1 host CPU); each shard's H2D streams in the
    # background while the next core is packed
    sc = _scratch()
    vpk = sc["vpk"]          # packed A_old rows (f32)
    dvec_pk = sc["dvec_pk"]  # packed dA rows (f32)
    xg = sc["xg"]
    xg_u8 = xg.view(np.uint8)
    delta_flat = delta.reshape(B, M)
    col_pack = IDX_PACK[None, :]
    shards = [None] * NCORES

    for c in range(NCORES):
        lo, hi = c * RB, (c + 1) * RB
        vp_c = fib[uid[lo:hi, None], col_pack]
        vp_c[:, PAD_COLS] = 0.0
        vpk[lo:hi] = vp_c
        xg_u8[c, 0] = LUT16_F8[
            np.multiply(vp_c, SA, dtype=np.float16).view(np.uint16)]
        # dvec_pk holds (up - lo); the 0.5 skew factor is folded into the
        # downstream constants (encode scale, norms, base add, hard path)
        dfc = delta_flat[lo:hi]
        dsub = dvec_pk[lo:hi]
        np.take(dfc, IU_UP_PACK, axis=1, out=dsub)
        dsub -= np.take(dfc, IU_LO_PACK, axis=1)
        dsub[:, PAD_COLS] = 0.0
        xg_u8[c, 1] = LUT16_F8[
            np.multiply(dsub, 0.5 * SD, dtype=np.float16).view(np.uint16)]
        if not trace:
            try:
                shards[c] = jax.device_put(xg[c].reshape(2 * RB, S), devices[c])
            except Exception:
                shards[c] = None
    _lap("pack+stream-put")

    fut_wpk = None
    if not trace:
        try:
            assert all(s is not None for s in shards)
            xd = jax.make_array_from_single_device_arrays(
                (NCORES * 2 * RB, S), sh, shards
            )
            outg = fn(xd, mup_d, msel_d, wz_d)   # async device pipeline
            fut_wpk = pool.submit(np.asarray, outg)  # network wait, no CPU
        except Exception:
            fut_wpk = None
        _lap("fn launch")

    # ---- overlap window: host work while device + D2H run ----
    # Frobenius sufficient conditions; rows they cannot settle go through
    # the exact host path (norms taken before the in-place base add below)
    fro_A = np.sqrt(2.0) * np.linalg.norm(vpk, axis=1)
    fro_dAe = 0.5 * ETA * np.sqrt(2.0) * np.linalg.norm(dvec_pk, axis=1)
    hard = ((RADIUS - fro_A) < fro_dAe + 1e-6) | (
        fro_A + fro_dAe + fro_A * fro_dAe > RADIUS - 1e-6
    )
    if hard.any():
        V_h = fib[uid[np.nonzero(hard)[0]]]
        hard_w = _exact_w_rows(V_h, 0.5 * dvec_pk[hard][:, INV])
    else:
        V_h = hard_w = None

    # base rows in packed space: vpk += eta * 0.5 * (up - lo) (in place)
    dvec_pk *= 0.5 * ETA
    vpk += dvec_pk

    out = fib.copy()
    _lap("overlap work")

    if trace:
        if "nc" not in _NC_CACHE:
            _NC_CACHE["nc"] = _build_nc()
        in_maps = []
        xgc = xg.reshape(NCORES, 2 * RB, S)
        for c in range(NCORES):
            in_maps.append({
                "x": xgc[c], "mup": MASK_UP, "msel": MASK_SEL,
            })
        res = bass_utils.run_bass_kernel_spmd(
            _NC_CACHE["nc"], in_maps, core_ids=list(range(NCORES)), trace=True,
        )
        LAST_EXEC_NS = res.exec_time_ns
        wpk = np.concatenate(
            [np.asarray(res.results[c]["wp"]) for c in range(NCORES)], axis=0
        )
    else:
        wpk = None
        if fut_wpk is not None:
            try:
                wpk = fut_wpk.result()
            except Exception:
                wpk = None
        if wpk is None:
            # device hiccup (e.g. exec-unit recovery): retry synchronously
            last_err = None
            for attempt in range(3):
                try:
                    _time.sleep(2.0 * attempt)
                    sh2 = [
                        jax.device_put(xg[c].reshape(2 * RB, S), devices[c])
                        for c in range(NCORES)
                    ]
                    xd2 = jax.make_array_from_single_device_arrays(
                        (NCORES * 2 * RB, S), sh, sh2
                    )
                    wpk = np.asarray(fn(xd2, mup_d, msel_d, wz_d))
                    break
                except Exception as e:
                    last_err = e
                    wpk = None
            if wpk is None:
                raise last_err
        LAST_EXEC_NS = None
        _lap("wait+D2H")

    # rows_new(packed) = base + scaled bracket; unpack via INV and scatter
    np.take(
        LUT2_SCALED, np.ascontiguousarray(wpk).view(np.uint16), axis=0,
        out=sc["brk"].reshape(B, S // 2, 2),
    )
    vpk += sc["brk"]
    np.take(vpk, INV, axis=1, out=sc["rows"])
    out[uid] = sc["rows"]
    _lap("post")

    if hard_w is not None:
        out[uid[hard]] = V_h + hard_w
    return out


_start_warmup()
